# revision 15
# baseline (speedup 1.0000x reference)
"""GAT (2-layer, PyG GATConv) Trainium2 kernel over 8 NeuronCores.

Strategy (v2):
  - Nodes are degree-sorted and dealt round-robin to 8 cores (dst-sharding);
    each core owns a contiguous row range of the permuted node table.
  - Phase 1 (sharded): each core computes h1/alpha1 only for its own NPC
    nodes (49 matmul tiles from a [128, NPC] xT shard), packs bf16 rows
    (512 B: h bf16[128] | asrc f32[4] | adst f32[4] | zeros), then one
    AllGather produces the full row table hext on every core.
  - Edge phase (dst-sharded): per 128-dst-node chunk, batched dma_gather of
    src rows (two gathers per chunk since dma_gather indices are int16:
    the table is split in two halves), attention weights via
    w = max(exp(t), exp(0.2 t)) (== exp(leaky_relu(t))), per-edge multiply
    on DVE, segment-sum via one strided DVE tensor_reduce over the slot
    axis (no per-slot matmuls).
  - Layer-2 projection fused per chunk; h2 shards AllGathered, then the same
    edge machinery runs for layer 2 (f32 rows), followed by fused
    log_softmax.
  - Host work is fully vectorized; attention-weight fusions (W@A) are
    precomputed on host; the int16 gather-index band is sent once
    ([16, TOTCOL]) and replicated to 128 partitions on device.
"""
import os
import sys

os.environ.setdefault("NEURON_RT_RESET_CORES", "1")
sys.path.insert(0, "/opt/trn_rl_repo")
sys.path.insert(0, "/root/.axon_site/_ro/trn_rl_repo")

import numpy as np
import ml_dtypes
import threading

_warm = {}


def _warm_jax():
    try:
        import jax

        _warm["devices"] = jax.devices()
    except Exception as e:  # pragma: no cover
        _warm["jax_err"] = e


def _warm_isa():
    try:
        from concourse.isa import get_isa

        get_isa("TRN2")
    except Exception as e:  # pragma: no cover
        _warm["isa_err"] = e


if os.environ.get("K_NOWARM", "0") == "1":
    _warm_threads = []
else:
    _warm_threads = [
        threading.Thread(target=_warm_jax, daemon=True),
        threading.Thread(target=_warm_isa, daemon=True),
    ]
    for _t in _warm_threads:
        _t.start()


def _default_cfg():
    return dict(N=50000, E=800000, F=128, H=4, C=32, CLASSES=40, NCORES=8)


def _host_tables(edge_index, cfg):
    """Build permutation + per-core gather-index band (vectorized)."""
    N, NCORES = cfg["N"], cfg["NCORES"]
    src0 = np.asarray(edge_index[0], dtype=np.int64)
    dst0 = np.asarray(edge_index[1], dtype=np.int64)
    E = src0.shape[0]

    NPC = int(np.ceil(np.ceil(N / NCORES) / 128) * 128)  # rows per core shard
    CHUNKS = NPC // 128
    NTOT = NPC * NCORES
    HALF = NTOT // 2
    assert HALF < 32767, "int16 index space exceeded"

    deg = np.bincount(dst0, minlength=N)
    rank_order = np.argsort(-deg, kind="stable")  # orig ids by rank
    rank_of = np.empty(N, dtype=np.int64)
    rank_of[rank_order] = np.arange(N)
    core_of = rank_of % NCORES
    local_of = rank_of // NCORES
    row_of = core_of * NPC + local_of  # permuted row id per orig node
    real_per_core = np.bincount(core_of, minlength=NCORES)
    assert real_per_core.max() < NPC, "need at least one junk row per shard"
    PAD_LOCAL = NPC - 1  # junk row in every shard; rows k*NPC+PAD_LOCAL

    src_r = row_of[src0]
    dst_r = row_of[dst0]
    core = dst_r // NPC
    ld = dst_r % NPC
    chunk = ld // 128
    lane = ld % 128
    st = (src_r >= HALF).astype(np.int64)

    # group edges by (core, chunk, stream, lane); slot = position in group
    key = ((core * CHUNKS + chunk) * 2 + st) * 128 + lane
    order = np.argsort(key, kind="stable")
    k_sorted = key[order]
    is_new = np.r_[True, k_sorted[1:] != k_sorted[:-1]]
    grp_start = np.maximum.accumulate(np.where(is_new, np.arange(E), 0))
    slot = np.arange(E) - grp_start

    cnt = np.bincount(key, minlength=NCORES * CHUNKS * 2 * 128)
    cnt4 = cnt.reshape(NCORES, CHUNKS, 2, 128)
    S = cnt4.max(axis=(0, 3))  # [CHUNKS, 2] edge-slot count per chunk/stream

    # flat band layout: for each (c, t) in c-major order, (S+1)*8 columns;
    # gather idx for (slot s, lane l) sits at (partition l%16,
    # column col_off + s*8 + l//16)  [i = s*128+l = col*16 + p]
    ns_flat = (S + 1).reshape(-1)
    col_off_flat = np.zeros(CHUNKS * 2, dtype=np.int64)
    np.cumsum(ns_flat[:-1] * 8, out=col_off_flat[1:])
    TOTCOL = int((ns_flat * 8).sum())

    band = np.full((NCORES, 16, TOTCOL), PAD_LOCAL, dtype=np.int16)
    # slot-0 entries: dst node's own row (if in the stream's half, else PAD)
    k_ = np.arange(NCORES)[:, None, None, None]
    c_ = np.arange(CHUNKS)[None, :, None, None]
    t_ = np.arange(2)[None, None, :, None]
    l_ = np.arange(128)[None, None, None, :]
    rows = k_ * NPC + c_ * 128 + l_
    base = t_ * HALF
    val0 = np.where((rows >= base) & (rows < base + HALF), rows - base, PAD_LOCAL)
    col0 = col_off_flat.reshape(1, CHUNKS, 2, 1) + l_ // 16
    kb, p0, colb, val0 = np.broadcast_arrays(k_, l_ % 16, col0, val0)
    band[kb, p0, colb] = val0.astype(np.int16)
    # edge entries
    e_core = core[order]
    e_ct = (chunk[order] * 2 + st[order])
    e_l = lane[order]
    e_idx = (src_r[order] - st[order] * HALF).astype(np.int16)
    e_col = col_off_flat[e_ct] + (slot + 1) * 8 + e_l // 16
    band[e_core, e_l % 16, e_col] = e_idx

    col_off = {}
    for c in range(CHUNKS):
        for t in range(2):
            col_off[(c, t)] = int(col_off_flat[c * 2 + t])

    meta = dict(NPC=NPC, CHUNKS=CHUNKS, NTOT=NTOT, HALF=HALF,
                PAD_LOCAL=PAD_LOCAL, S=S, col_off=col_off, row_of=row_of,
                TOTCOL=TOTCOL)
    return band, meta


def _build_program(cfg, meta):
    import concourse.bass as bass
    import concourse.bacc as bacc
    import concourse.tile as tile
    from concourse import mybir
    from concourse.masks import make_identity

    GCHUNK = int(os.environ.get("K_GCHUNK", "8"))
    F, H, C, CLASSES, NCORES = cfg["F"], cfg["H"], cfg["C"], cfg["CLASSES"], cfg["NCORES"]
    HC = H * C
    NPC, CHUNKS, NTOT, HALF = meta["NPC"], meta["CHUNKS"], meta["NTOT"], meta["HALF"]
    S = meta["S"]
    col_off = meta["col_off"]
    TOTCOL = meta["TOTCOL"]
    PAD_LOCAL = meta["PAD_LOCAL"]
    P = 128
    RB1 = 256  # bf16 cols per L1 row (512 B): h bf16[0:128], f32 cols 64:68 asrc, 68:72 adst
    RB2 = 64   # f32 cols per L2 row (256 B): h2[0:40], 40 asrc2, 41 adst2
    f32, bf16, i16 = mybir.dt.float32, mybir.dt.bfloat16, mybir.dt.int16
    EPS = 1e-16

    nc = bacc.Bacc(num_devices=NCORES)
    t_xT = nc.declare_dram_parameter("xT", [P, NPC], bf16, isOutput=False)
    t_W1e = nc.declare_dram_parameter("W1e", [F, HC + 2 * H], bf16, isOutput=False)
    t_W2e = nc.declare_dram_parameter("W2e", [HC, CLASSES + 2], f32, isOutput=False)
    t_B1 = nc.declare_dram_parameter("B1", [P, HC], f32, isOutput=False)
    t_B2 = nc.declare_dram_parameter("B2", [P, CLASSES], f32, isOutput=False)
    t_idx = nc.declare_dram_parameter("idx", [16, TOTCOL], i16, isOutput=False)
    o_out = nc.declare_dram_parameter("out", [NPC, CLASSES], bf16, isOutput=True)

    with tile.TileContext(nc) as tc:
        with (
            tc.tile_pool(name="persist", bufs=1) as pp,
            tc.tile_pool(name="dram", bufs=1, space="DRAM") as dram,
        ):
            hloc = dram.tile([NPC, RB1], bf16)
            hext = dram.tile([NTOT, RB1], bf16, addr_space="Shared")
            h2sh = dram.tile([NPC, RB2], f32)
            h2full = dram.tile([NTOT, RB2], f32, addr_space="Shared")

            sb_idx = pp.tile([P, TOTCOL], i16)
            for r in range(8):
                nc.sync.dma_start(sb_idx[16 * r : 16 * (r + 1), :], t_idx[:])

            w1e = pp.tile([F, HC + 2 * H], bf16)
            nc.sync.dma_start(w1e[:], t_W1e[:])
            w2e = pp.tile([HC, CLASSES + 2], f32)
            nc.sync.dma_start(w2e[:], t_W2e[:])
            sb_B1 = pp.tile([P, HC], f32)
            nc.sync.dma_start(sb_B1[:], t_B1[:])
            sb_B2 = pp.tile([P, CLASSES], f32)
            nc.sync.dma_start(sb_B2[:], t_B2[:])

            ident_f = pp.tile([P, P], f32)
            make_identity(nc, ident_f[:])
            neg_const = pp.tile([1, 4], f32)
            nc.vector.memset(neg_const[:], -1e4)

            x2T_all = pp.tile([P, NPC], f32)  # persistent layer-2 input (transposed)

            # ---------------- phase 1: h1 rows for OWN shard, then AllGather
            with (
                tc.tile_pool(name="p1x", bufs=3) as p1x,
                tc.tile_pool(name="p1h", bufs=3) as p1h,
                tc.tile_pool(name="p1ps", bufs=2, space="PSUM") as p1ps,
            ):
                for t in range(CHUNKS):
                    xt = p1x.tile([P, P], bf16)
                    nc.sync.dma_start(xt[:], t_xT[:, t * P : (t + 1) * P])
                    ph = p1ps.tile([P, HC + 2 * H], f32)
                    nc.tensor.matmul(out=ph[:], lhsT=xt[:], rhs=w1e[:], start=True, stop=True)
                    hx = p1h.tile([P, RB1], bf16)
                    nc.gpsimd.memset(hx[:, 2 * (64 + 2 * H) : RB1], 0.0)
                    if t % 2 == 0:
                        nc.scalar.copy(hx[:, 0:HC], ph[:, 0:HC])
                    else:
                        nc.vector.tensor_copy(hx[:, 0:HC], ph[:, 0:HC])
                    hxf = hx[:].bitcast(f32)
                    nc.vector.tensor_copy(hxf[:, 64 : 64 + 2 * H], ph[:, HC : HC + 2 * H])
                    nc.sync.dma_start(hloc[t * P : (t + 1) * P, :], hx[:])
                # patch own pad row's asrc = -1e4 (one pad row per shard)
                hf = hloc[:].bitcast(f32)
                nc.sync.dma_start(hf[PAD_LOCAL : PAD_LOCAL + 1, 64:68], neg_const[:1, :4])

            nc.gpsimd.collective_compute(
                "AllGather",
                mybir.AluOpType.bypass,
                replica_groups=[list(range(NCORES))],
                ins=[hloc.opt()],
                outs=[hext.opt()],
            )

            # ---------------- layer-1 edge phase + layer-2 projection -----
            with (
                tc.tile_pool(name="e1g", bufs=2) as e1g,
                tc.tile_pool(name="e1w", bufs=2) as e1w,
                tc.tile_pool(name="e1t", bufs=2) as e1t,
                tc.tile_pool(name="e1o", bufs=2) as e1o,
                tc.tile_pool(name="e1ps2", bufs=1, space="PSUM") as e1ps2,
            ):
                for c in range(CHUNKS):
                    SA, SB = int(S[c, 0]), int(S[c, 1])
                    g = []
                    for t, Sn in ((0, SA), (1, SB)):
                        gt = e1g.tile([P, (Sn + 1) * RB1], bf16, tag=f"g{t}")
                        off = col_off[(c, t)]
                        for s0 in range(0, Sn + 1, GCHUNK):
                            s1 = min(s0 + GCHUNK, Sn + 1)
                            nc.gpsimd.dma_gather(
                                out_ap=gt[:, s0 * RB1 : s1 * RB1].rearrange(
                                    "p (s r) -> p s r", r=RB1
                                ),
                                in_ap=hext[t * HALF : (t + 1) * HALF, :],
                                idxs_ap=sb_idx[:, off + s0 * 8 : off + s1 * 8],
                                num_idxs=(s1 - s0) * P,
                                num_idxs_reg=(s1 - s0) * P,
                                elem_size=RB1,
                            )
                        g.append(gt)
                    gA = g[0][:].bitcast(f32).rearrange("p (s r) -> p s r", r=RB1 // 2)
                    gB = g[1][:].bitcast(f32).rearrange("p (s r) -> p s r", r=RB1 // 2)

                    adst = e1w.tile([P, H], f32)
                    nc.vector.tensor_tensor(
                        out=adst[:], in0=gA[:, 0, 68:72], in1=gB[:, 0, 68:72],
                        op=mybir.AluOpType.add,
                    )
                    ST = SA + SB
                    t_all = e1w.tile([P, ST * H], f32)
                    nc.vector.tensor_tensor(
                        out=t_all[:, : SA * H].rearrange("p (s h) -> p s h", h=H),
                        in0=gA[:, 1:, 64:68],
                        in1=adst[:].unsqueeze(1).to_broadcast((P, SA, H)),
                        op=mybir.AluOpType.add,
                    )
                    nc.vector.tensor_tensor(
                        out=t_all[:, SA * H :].rearrange("p (s h) -> p s h", h=H),
                        in0=gB[:, 1:, 64:68],
                        in1=adst[:].unsqueeze(1).to_broadcast((P, SB, H)),
                        op=mybir.AluOpType.add,
                    )
                    e1_t = e1w.tile([P, ST * H], f32)
                    nc.scalar.activation(e1_t[:], t_all[:], mybir.ActivationFunctionType.Exp)
                    e2_t = e1w.tile([P, ST * H], f32)
                    nc.scalar.activation(
                        e2_t[:], t_all[:], mybir.ActivationFunctionType.Exp, scale=0.2
                    )
                    w_all = e1w.tile([P, ST * H], f32)
                    nc.vector.tensor_tensor(
                        out=w_all[:], in0=e1_t[:], in1=e2_t[:], op=mybir.AluOpType.max
                    )
                    den = e1w.tile([P, H], f32)
                    nc.vector.tensor_reduce(
                        out=den[:],
                        in_=w_all[:].rearrange("p (s h) -> p h s", h=H),
                        axis=mybir.AxisListType.X,
                        op=mybir.AluOpType.add,
                    )
                    wb = e1w.tile([P, ST * H], bf16)
                    nc.vector.tensor_copy(wb[:], w_all[:])

                    tmp = e1t.tile([P, ST * HC], bf16)
                    nc.vector.tensor_tensor(
                        out=tmp[:, : SA * HC].rearrange("p (s h c) -> p s h c", h=H, c=C),
                        in0=g[0][:].rearrange("p (s r) -> p s r", r=RB1)[:, 1:, 0:HC]
                        .rearrange("p s (h c) -> p s h c", h=H),
                        in1=wb[:, : SA * H].rearrange("p (s h) -> p s h", h=H)
                        .unsqueeze(3).to_broadcast((P, SA, H, C)),
                        op=mybir.AluOpType.mult,
                    )
                    nc.vector.tensor_tensor(
                        out=tmp[:, SA * HC :].rearrange("p (s h c) -> p s h c", h=H, c=C),
                        in0=g[1][:].rearrange("p (s r) -> p s r", r=RB1)[:, 1:, 0:HC]
                        .rearrange("p s (h c) -> p s h c", h=H),
                        in1=wb[:, SA * H :].rearrange("p (s h) -> p s h", h=H)
                        .unsqueeze(3).to_broadcast((P, SB, H, C)),
                        op=mybir.AluOpType.mult,
                    )
                    acc = e1t.tile([P, HC], f32)
                    nc.vector.tensor_reduce(
                        out=acc[:],
                        in_=tmp[:].rearrange("p (s hc) -> p hc s", hc=HC),
                        axis=mybir.AxisListType.X,
                        op=mybir.AluOpType.add,
                    )
                    den_e = e1w.tile([P, H], f32)
                    nc.vector.tensor_scalar(
                        out=den_e[:], in0=den[:], scalar1=EPS, scalar2=None,
                        op0=mybir.AluOpType.add,
                    )
                    den_r = e1w.tile([P, H], f32)
                    nc.vector.reciprocal(den_r[:], den_e[:])
                    x2 = e1o.tile([P, HC], f32)
                    nc.vector.tensor_tensor(
                        out=x2[:].rearrange("p (h c) -> p h c", h=H),
                        in0=acc[:].rearrange("p (h c) -> p h c", h=H),
                        in1=den_r[:].unsqueeze(2).to_broadcast((P, H, C)),
                        op=mybir.AluOpType.mult,
                    )
                    nc.vector.tensor_tensor(
                        out=x2[:], in0=x2[:], in1=sb_B1[:], op=mybir.AluOpType.add
                    )
                    x2r = e1o.tile([P, HC], f32)
                    nc.scalar.activation(x2r[:], x2[:], mybir.ActivationFunctionType.Relu)

                    # layer-2 projection for this chunk
                    xt2 = e1ps2.tile([P, P], f32)
                    nc.tensor.transpose(out=xt2[:], in_=x2r[:], identity=ident_f[:])
                    nc.vector.tensor_copy(x2T_all[:, c * P : (c + 1) * P], xt2[:])
                    h2p = e1ps2.tile([P, CLASSES + 2], f32)
                    nc.tensor.matmul(
                        out=h2p[:], lhsT=x2T_all[:, c * P : (c + 1) * P], rhs=w2e[:],
                        start=True, stop=True,
                    )
                    hx2 = e1o.tile([P, RB2], f32)
                    nc.gpsimd.memset(hx2[:, CLASSES + 2 : RB2], 0.0)
                    nc.vector.tensor_copy(hx2[:, 0 : CLASSES + 2], h2p[:])
                    nc.sync.dma_start(h2sh[c * P : (c + 1) * P, :], hx2[:])

                # patch local pad row asrc2 = -1e4 (every core patches its own)
                nc.sync.dma_start(
                    h2sh[PAD_LOCAL : PAD_LOCAL + 1, CLASSES : CLASSES + 1],
                    neg_const[:1, :1],
                )

            # ---------------- AllGather h2full ----------------------------
            nc.gpsimd.collective_compute(
                "AllGather",
                mybir.AluOpType.bypass,
                replica_groups=[list(range(NCORES))],
                ins=[h2sh.opt()],
                outs=[h2full.opt()],
            )

            # ---------------- layer-2 edge phase + log_softmax ------------
            with (
                tc.tile_pool(name="e2g", bufs=2) as e2g,
                tc.tile_pool(name="e2w", bufs=2) as e2w,
                tc.tile_pool(name="e2t", bufs=2) as e2t,
                tc.tile_pool(name="e2o", bufs=2) as e2o,
            ):
                for c in range(CHUNKS):
                    SA, SB = int(S[c, 0]), int(S[c, 1])
                    g = []
                    for t, Sn in ((0, SA), (1, SB)):
                        gt = e2g.tile([P, (Sn + 1) * RB2], f32, tag=f"g2{t}")
                        off = col_off[(c, t)]
                        for s0 in range(0, Sn + 1, GCHUNK):
                            s1 = min(s0 + GCHUNK, Sn + 1)
                            nc.gpsimd.dma_gather(
                                out_ap=gt[:, s0 * RB2 : s1 * RB2].rearrange(
                                    "p (s r) -> p s r", r=RB2
                                ),
                                in_ap=h2full[t * HALF : (t + 1) * HALF, :],
                                idxs_ap=sb_idx[:, off + s0 * 8 : off + s1 * 8],
                                num_idxs=(s1 - s0) * P,
                                num_idxs_reg=(s1 - s0) * P,
                                elem_size=RB2,
                            )
                        g.append(gt)
                    gA = g[0][:].rearrange("p (s r) -> p s r", r=RB2)
                    gB = g[1][:].rearrange("p (s r) -> p s r", r=RB2)

                    adst2 = e2w.tile([P, 1], f32)
                    nc.vector.tensor_tensor(
                        out=adst2[:], in0=gA[:, 0, 41:42], in1=gB[:, 0, 41:42],
                        op=mybir.AluOpType.add,
                    )
                    ST = SA + SB
                    t2 = e2w.tile([P, ST], f32)
                    nc.vector.tensor_tensor(
                        out=t2[:, :SA],
                        in0=gA[:, 1:, 40],
                        in1=adst2[:].to_broadcast((P, SA)),
                        op=mybir.AluOpType.add,
                    )
                    nc.vector.tensor_tensor(
                        out=t2[:, SA:],
                        in0=gB[:, 1:, 40],
                        in1=adst2[:].to_broadcast((P, SB)),
                        op=mybir.AluOpType.add,
                    )
                    e1_2 = e2w.tile([P, ST], f32)
                    nc.scalar.activation(e1_2[:], t2[:], mybir.ActivationFunctionType.Exp)
                    e2_2 = e2w.tile([P, ST], f32)
                    nc.scalar.activation(
                        e2_2[:], t2[:], mybir.ActivationFunctionType.Exp, scale=0.2
                    )
                    w2_all = e2w.tile([P, ST], f32)
                    nc.vector.tensor_tensor(
                        out=w2_all[:], in0=e1_2[:], in1=e2_2[:], op=mybir.AluOpType.max
                    )
                    den2 = e2w.tile([P, 1], f32)
                    nc.vector.tensor_reduce(
                        out=den2[:], in_=w2_all[:], axis=mybir.AxisListType.X,
                        op=mybir.AluOpType.add,
                    )
                    tmp2 = e2t.tile([P, ST * CLASSES], f32)
                    nc.vector.tensor_tensor(
                        out=tmp2[:, : SA * CLASSES].rearrange("p (s f) -> p s f", f=CLASSES),
                        in0=gA[:, 1:, 0:CLASSES],
                        in1=w2_all[:, :SA].unsqueeze(2).to_broadcast((P, SA, CLASSES)),
                        op=mybir.AluOpType.mult,
                    )
                    nc.vector.tensor_tensor(
                        out=tmp2[:, SA * CLASSES :].rearrange("p (s f) -> p s f", f=CLASSES),
                        in0=gB[:, 1:, 0:CLASSES],
                        in1=w2_all[:, SA:].unsqueeze(2).to_broadcast((P, SB, CLASSES)),
                        op=mybir.AluOpType.mult,
                    )
                    acc2 = e2t.tile([P, CLASSES], f32)
                    nc.vector.tensor_reduce(
                        out=acc2[:],
                        in_=tmp2[:].rearrange("p (s f) -> p f s", f=CLASSES),
                        axis=mybir.AxisListType.X,
                        op=mybir.AluOpType.add,
                    )
                    den2e = e2w.tile([P, 1], f32)
                    nc.vector.tensor_scalar(
                        out=den2e[:], in0=den2[:], scalar1=EPS, scalar2=None,
                        op0=mybir.AluOpType.add,
                    )
                    den2r = e2w.tile([P, 1], f32)
                    nc.vector.reciprocal(den2r[:], den2e[:])
                    o_pre = e2o.tile([P, CLASSES], f32)
                    nc.vector.tensor_tensor(
                        out=o_pre[:], in0=acc2[:],
                        in1=den2r[:].to_broadcast((P, CLASSES)),
                        op=mybir.AluOpType.mult,
                    )
                    nc.vector.tensor_tensor(
                        out=o_pre[:], in0=o_pre[:], in1=sb_B2[:], op=mybir.AluOpType.add
                    )
                    # log_softmax
                    nmax = e2w.tile([P, 1], f32)
                    nc.vector.tensor_reduce(
                        out=nmax[:], in_=o_pre[:], axis=mybir.AxisListType.X,
                        op=mybir.AluOpType.max, negate=True,
                    )
                    expt = e2w.tile([P, CLASSES], f32)
                    sumexp = e2w.tile([P, 1], f32)
                    nc.scalar.activation(
                        expt[:], o_pre[:], mybir.ActivationFunctionType.Exp,
                        bias=nmax[:, 0:1], accum_out=sumexp[:, 0:1],
                    )
                    lse = e2w.tile([P, 1], f32)
                    nc.scalar.activation(lse[:], sumexp[:], mybir.ActivationFunctionType.Ln)
                    sh = e2w.tile([P, 1], f32)
                    nc.vector.tensor_tensor(
                        out=sh[:], in0=nmax[:], in1=lse[:], op=mybir.AluOpType.subtract
                    )
                    o_f = e2o.tile([P, CLASSES], bf16)
                    nc.scalar.activation(
                        o_f[:], o_pre[:], mybir.ActivationFunctionType.Identity,
                        bias=sh[:, 0:1],
                    )
                    nc.sync.dma_start(o_out[c * P : (c + 1) * P, :], o_f[:])
    nc.finalize()
    return nc


def _kernel_impl(x, W1, a_src1, a_dst1, b1, W2, a_src2, a_dst2, b2, edge_index, cfg):
    N, F, H, C, CLASSES, NCORES = (
        cfg["N"], cfg["F"], cfg["H"], cfg["C"], cfg["CLASSES"], cfg["NCORES"]
    )
    HC = H * C
    x = np.asarray(x, dtype=np.float32)
    band, meta = _host_tables(np.asarray(edge_index), cfg)
    NPC, NTOT = meta["NPC"], meta["NTOT"]
    row_of = meta["row_of"]

    xp = np.zeros((NTOT, F), dtype=np.float32)
    xp[row_of] = x
    xT = np.ascontiguousarray(xp.T)

    a_src1 = np.asarray(a_src1, np.float32)
    a_dst1 = np.asarray(a_dst1, np.float32)
    A1 = np.zeros((HC, 2 * H), dtype=np.float32)
    for h in range(H):
        A1[h * C : (h + 1) * C, h] = a_src1[h]
        A1[h * C : (h + 1) * C, H + h] = a_dst1[h]
    A2 = np.stack(
        [np.asarray(a_src2, np.float32)[0], np.asarray(a_dst2, np.float32)[0]], axis=1
    )
    W1 = np.asarray(W1, np.float32)
    W2 = np.asarray(W2, np.float32)
    W1e = np.concatenate([W1, W1 @ A1], axis=1).astype(ml_dtypes.bfloat16)
    W2e = np.concatenate([W2, W2 @ A2], axis=1)  # [HC, CLASSES + 2]
    B1 = np.ascontiguousarray(
        np.broadcast_to(np.asarray(b1, np.float32), (128, HC))
    )
    B2 = np.ascontiguousarray(
        np.broadcast_to(np.asarray(b2, np.float32), (128, CLASSES))
    )

    for _t in _warm_threads:
        _t.join()
    from concourse.bass_utils import run_bass_kernel_spmd

    nc = _build_program(cfg, meta)
    common = dict(W1e=W1e, W2e=W2e, B1=B1, B2=B2)
    xTb = xT.astype(ml_dtypes.bfloat16)
    in_maps = [
        dict(
            common,
            idx=band[k],
            xT=np.ascontiguousarray(xTb[:, k * NPC : (k + 1) * NPC]),
        )
        for k in range(NCORES)
    ]
    res = run_bass_kernel_spmd(nc, in_maps, list(range(NCORES)))
    outs = np.concatenate(
        [np.asarray(res.results[k]["out"]) for k in range(NCORES)], axis=0
    ).astype(np.float32)
    return np.ascontiguousarray(outs[row_of])


def kernel(x, W1, a_src1, a_dst1, b1, W2, a_src2, a_dst2, b2, edge_index):
    return _kernel_impl(
        x, W1, a_src1, a_dst1, b1, W2, a_src2, a_dst2, b2, edge_index, _default_cfg()
    )


# revision 30
# speedup vs baseline: 1.4741x; 1.4741x over previous
"""GAT (2-layer, PyG GATConv) Trainium2 kernel over 8 NeuronCores.

Strategy (v2):
  - Nodes are degree-sorted and dealt round-robin to 8 cores (dst-sharding);
    each core owns a contiguous row range of the permuted node table.
  - Phase 1 (sharded): each core computes h1/alpha1 only for its own NPC
    nodes (49 matmul tiles from a [128, NPC] xT shard), packs bf16 rows
    (512 B: h bf16[128] | asrc f32[4] | adst f32[4] | zeros), then one
    AllGather produces the full row table hext on every core.
  - Edge phase (dst-sharded): per 128-dst-node chunk, batched dma_gather of
    src rows (two gathers per chunk since dma_gather indices are int16:
    the table is split in two halves), attention weights via
    w = max(exp(t), exp(0.2 t)) (== exp(leaky_relu(t))), per-edge multiply
    on DVE, segment-sum via one strided DVE tensor_reduce over the slot
    axis (no per-slot matmuls).
  - Layer-2 projection fused per chunk; h2 shards AllGathered, then the same
    edge machinery runs for layer 2 (f32 rows), followed by fused
    log_softmax.
  - Host work is fully vectorized; attention-weight fusions (W@A) are
    precomputed on host; the int16 gather-index band is sent once
    ([16, TOTCOL]) and replicated to 128 partitions on device.
"""
import os
import sys

os.environ.setdefault("NEURON_RT_RESET_CORES", "1")
sys.path.insert(0, "/opt/trn_rl_repo")
sys.path.insert(0, "/root/.axon_site/_ro/trn_rl_repo")

import numpy as np
import ml_dtypes
import threading

_warm = {}


def _warm_jax():
    try:
        import jax

        _warm["devices"] = jax.devices()
    except Exception as e:  # pragma: no cover
        _warm["jax_err"] = e


_isa_done = threading.Event()


def _warm_isa():
    try:
        from concourse.isa import get_isa

        get_isa("TRN2")
        import concourse.bass_utils  # noqa: F401  (preload for main thread)
        import concourse.bacc  # noqa: F401
        import concourse.tile  # noqa: F401
        import concourse.masks  # noqa: F401
    except Exception as e:  # pragma: no cover
        _warm["isa_err"] = e
    finally:
        _isa_done.set()


def _warm_exec():
    """After jax + ISA are up, run a tiny AllGather program once so the
    per-process PJRT/NRT/global-comm setup happens off the critical path."""
    try:
        _warm_jax()
        _isa_done.wait(timeout=120)
        import concourse.bacc as bacc
        import concourse.tile as tile
        from concourse import mybir
        from concourse.bass_utils import run_bass_kernel_spmd

        f32 = mybir.dt.float32
        nc = bacc.Bacc(num_devices=8)
        t_in = nc.declare_dram_parameter("win", [128, 16], f32, isOutput=False)
        t_out = nc.declare_dram_parameter("wout", [128, 16], f32, isOutput=True)
        with tile.TileContext(nc) as tc:
            with (
                tc.tile_pool(name="wsb", bufs=1) as sb,
                tc.tile_pool(name="wdr", bufs=1, space="DRAM") as dr,
            ):
                gin = dr.tile([16, 16], f32)
                gout = dr.tile([128, 16], f32, addr_space="Shared")
                a = sb.tile([128, 16], f32)
                nc.sync.dma_start(a[:], t_in[:])
                nc.sync.dma_start(gin[:], a[0:16, :])
                nc.gpsimd.collective_compute(
                    "AllGather",
                    mybir.AluOpType.bypass,
                    replica_groups=[list(range(8))],
                    ins=[gin.opt()],
                    outs=[gout.opt()],
                )
                b = sb.tile([128, 16], f32)
                nc.sync.dma_start(b[:], gout[:])
                nc.sync.dma_start(t_out[:], b[:])
        nc.finalize()
        z = np.zeros((128, 16), np.float32)
        run_bass_kernel_spmd(nc, [dict(win=z)] * 8, list(range(8)))
        _warm["exec"] = True
    except Exception as e:  # pragma: no cover
        _warm["exec_err"] = e


if os.environ.get("K_NOWARM", "0") == "1":
    _warm_threads = []
    _isa_done.set()
else:
    _warm_threads = [
        threading.Thread(target=_warm_jax, daemon=True),
        threading.Thread(target=_warm_isa, daemon=True),
    ]
    for _t in _warm_threads:
        _t.start()


def _default_cfg():
    return dict(N=50000, E=800000, F=128, H=4, C=32, CLASSES=40, NCORES=8)


def _host_tables(edge_index, cfg):
    """Build permutation + per-core gather-index band (vectorized)."""
    N, NCORES = cfg["N"], cfg["NCORES"]
    src0 = np.asarray(edge_index[0], dtype=np.int64)
    dst0 = np.asarray(edge_index[1], dtype=np.int64)
    E = src0.shape[0]

    NPC = int(np.ceil(np.ceil(N / NCORES) / 128) * 128)  # rows per core shard
    CHUNKS = NPC // 128
    NTOT = NPC * NCORES
    HALF = NTOT // 2
    assert HALF < 32767, "int16 index space exceeded"

    deg = np.bincount(dst0, minlength=N)
    rank_order = np.argsort(-deg, kind="stable")  # orig ids by rank
    rank_of = np.empty(N, dtype=np.int64)
    rank_of[rank_order] = np.arange(N)
    core_of = rank_of % NCORES
    local_of = rank_of // NCORES
    row_of = core_of * NPC + local_of  # permuted row id per orig node
    real_per_core = np.bincount(core_of, minlength=NCORES)
    assert real_per_core.max() < NPC, "need at least one junk row per shard"
    PAD_LOCAL = NPC - 1  # junk row in every shard; rows k*NPC+PAD_LOCAL

    src_r = row_of[src0]
    dst_r = row_of[dst0]
    core = dst_r // NPC
    ld = dst_r % NPC
    chunk = ld // 128
    lane = ld % 128
    st = (src_r >= HALF).astype(np.int64)

    # group edges by (core, chunk, stream, lane); slot = position in group
    key = (((core * CHUNKS + chunk) * 2 + st) * 128 + lane).astype(np.int32)
    order = np.argsort(key, kind="stable")
    k_sorted = key[order]
    is_new = np.r_[True, k_sorted[1:] != k_sorted[:-1]]
    grp_start = np.maximum.accumulate(np.where(is_new, np.arange(E), 0))
    slot = np.arange(E) - grp_start

    cnt = np.bincount(key, minlength=NCORES * CHUNKS * 2 * 128)
    cnt4 = cnt.reshape(NCORES, CHUNKS, 2, 128)
    S = cnt4.max(axis=(0, 3))  # [CHUNKS, 2] edge-slot count per chunk/stream

    # flat band layout: for each (c, t) in c-major order, (S+1)*8 columns;
    # gather idx for (slot s, lane l) sits at (partition l%16,
    # column col_off + s*8 + l//16)  [i = s*128+l = col*16 + p]
    ns_flat = (S + 1).reshape(-1)
    col_off_flat = np.zeros(CHUNKS * 2, dtype=np.int64)
    np.cumsum(ns_flat[:-1] * 8, out=col_off_flat[1:])
    TOTCOL = int((ns_flat * 8).sum())

    band = np.full((NCORES, 16, TOTCOL), PAD_LOCAL, dtype=np.int16)
    # slot-0 entries: dst node's own row (if in the stream's half, else PAD)
    k_ = np.arange(NCORES)[:, None, None, None]
    c_ = np.arange(CHUNKS)[None, :, None, None]
    t_ = np.arange(2)[None, None, :, None]
    l_ = np.arange(128)[None, None, None, :]
    rows = k_ * NPC + c_ * 128 + l_
    base = t_ * HALF
    val0 = np.where((rows >= base) & (rows < base + HALF), rows - base, PAD_LOCAL)
    col0 = col_off_flat.reshape(1, CHUNKS, 2, 1) + l_ // 16
    kb, p0, colb, val0 = np.broadcast_arrays(k_, l_ % 16, col0, val0)
    band[kb, p0, colb] = val0.astype(np.int16)
    # edge entries
    e_core = core[order]
    e_ct = (chunk[order] * 2 + st[order])
    e_l = lane[order]
    e_idx = (src_r[order] - st[order] * HALF).astype(np.int16)
    e_col = col_off_flat[e_ct] + (slot + 1) * 8 + e_l // 16
    band[e_core, e_l % 16, e_col] = e_idx

    col_off = {}
    for c in range(CHUNKS):
        for t in range(2):
            col_off[(c, t)] = int(col_off_flat[c * 2 + t])

    meta = dict(NPC=NPC, CHUNKS=CHUNKS, NTOT=NTOT, HALF=HALF,
                PAD_LOCAL=PAD_LOCAL, S=S, col_off=col_off, row_of=row_of,
                TOTCOL=TOTCOL)
    return band, meta


def _build_program(cfg, meta):
    import concourse.bass as bass
    import concourse.bacc as bacc
    import concourse.tile as tile
    from concourse import mybir
    from concourse.masks import make_identity

    GCHUNK = int(os.environ.get("K_GCHUNK", "8"))
    F, H, C, CLASSES, NCORES = cfg["F"], cfg["H"], cfg["C"], cfg["CLASSES"], cfg["NCORES"]
    HC = H * C
    NPC, CHUNKS, NTOT, HALF = meta["NPC"], meta["CHUNKS"], meta["NTOT"], meta["HALF"]
    S = meta["S"]
    col_off = meta["col_off"]
    TOTCOL = meta["TOTCOL"]
    PAD_LOCAL = meta["PAD_LOCAL"]
    P = 128
    RB1 = 256  # bf16 cols per L1 row (512 B): h bf16[0:128], f32 cols 64:68 asrc, 68:72 adst
    RB2 = 64   # f32 cols per L2 row (256 B): h2[0:40], 40 asrc2, 41 adst2
    f32, bf16, i16 = mybir.dt.float32, mybir.dt.bfloat16, mybir.dt.int16
    EPS = 1e-16

    nc = bacc.Bacc(num_devices=NCORES)
    t_xT = nc.declare_dram_parameter("xT", [P, NPC], bf16, isOutput=False)
    t_W1e = nc.declare_dram_parameter("W1e", [F, HC + 2 * H], bf16, isOutput=False)
    t_W2e = nc.declare_dram_parameter("W2e", [HC, CLASSES + 2], f32, isOutput=False)
    t_B1 = nc.declare_dram_parameter("B1", [P, HC], f32, isOutput=False)
    t_B2 = nc.declare_dram_parameter("B2", [P, CLASSES], f32, isOutput=False)
    t_idx = nc.declare_dram_parameter("idx", [16, TOTCOL], i16, isOutput=False)
    o_out = nc.declare_dram_parameter("out", [NPC, CLASSES], bf16, isOutput=True)

    with tile.TileContext(nc) as tc:
        with (
            tc.tile_pool(name="persist", bufs=1) as pp,
            tc.tile_pool(name="dram", bufs=1, space="DRAM") as dram,
        ):
            hloc = dram.tile([NPC, RB1], bf16)
            hext = dram.tile([NTOT, RB1], bf16, addr_space="Shared")
            h2sh = dram.tile([NPC, RB2], f32)
            h2full = dram.tile([NTOT, RB2], f32, addr_space="Shared")

            sb_idx = pp.tile([P, TOTCOL], i16)
            for r in range(8):
                nc.sync.dma_start(sb_idx[16 * r : 16 * (r + 1), :], t_idx[:])

            w1e = pp.tile([F, HC + 2 * H], bf16)
            nc.sync.dma_start(w1e[:], t_W1e[:])
            w2e = pp.tile([HC, CLASSES + 2], f32)
            nc.sync.dma_start(w2e[:], t_W2e[:])
            sb_B1 = pp.tile([P, HC], f32)
            nc.sync.dma_start(sb_B1[:], t_B1[:])
            sb_B2 = pp.tile([P, CLASSES], f32)
            nc.sync.dma_start(sb_B2[:], t_B2[:])

            ident_f = pp.tile([P, P], f32)
            make_identity(nc, ident_f[:])
            neg_const = pp.tile([1, 4], f32)
            nc.vector.memset(neg_const[:], -1e4)

            x2T_all = pp.tile([P, NPC], f32)  # persistent layer-2 input (transposed)

            # ---------------- phase 1: h1 rows for OWN shard, then AllGather
            with (
                tc.tile_pool(name="p1x", bufs=3) as p1x,
                tc.tile_pool(name="p1h", bufs=3) as p1h,
                tc.tile_pool(name="p1ps", bufs=2, space="PSUM") as p1ps,
            ):
                for t in range(CHUNKS):
                    xt = p1x.tile([P, P], bf16)
                    nc.sync.dma_start(xt[:], t_xT[:, t * P : (t + 1) * P])
                    ph = p1ps.tile([P, HC + 2 * H], f32)
                    nc.tensor.matmul(out=ph[:], lhsT=xt[:], rhs=w1e[:], start=True, stop=True)
                    hx = p1h.tile([P, RB1], bf16, tag="hx")
                    if t < 3:  # pool rotates 3 bufs; zero the tail once per buf
                        nc.gpsimd.memset(hx[:, 2 * (64 + 2 * H) : RB1], 0.0)
                    if t % 2 == 0:
                        nc.scalar.copy(hx[:, 0:HC], ph[:, 0:HC])
                    else:
                        nc.vector.tensor_copy(hx[:, 0:HC], ph[:, 0:HC])
                    hxf = hx[:].bitcast(f32)
                    nc.vector.tensor_copy(hxf[:, 64 : 64 + 2 * H], ph[:, HC : HC + 2 * H])
                    nc.sync.dma_start(hloc[t * P : (t + 1) * P, :], hx[:])
                # patch own pad row's asrc = -1e4 (one pad row per shard)
                hf = hloc[:].bitcast(f32)
                nc.sync.dma_start(hf[PAD_LOCAL : PAD_LOCAL + 1, 64:68], neg_const[:1, :4])

            nc.gpsimd.collective_compute(
                "AllGather",
                mybir.AluOpType.bypass,
                replica_groups=[list(range(NCORES))],
                ins=[hloc.opt()],
                outs=[hext.opt()],
            )

            # ---------------- layer-1 edge phase + layer-2 projection -----
            with (
                tc.tile_pool(name="e1g", bufs=2) as e1g,
                tc.tile_pool(name="e1w", bufs=2) as e1w,
                tc.tile_pool(name="e1t", bufs=2) as e1t,
                tc.tile_pool(name="e1o", bufs=2) as e1o,
                tc.tile_pool(name="e1ps2", bufs=1, space="PSUM") as e1ps2,
            ):
                for c in range(CHUNKS):
                    SA, SB = int(S[c, 0]), int(S[c, 1])
                    g = []
                    for t, Sn in ((0, SA), (1, SB)):
                        gt = e1g.tile([P, (Sn + 1) * RB1], bf16, tag=f"g{t}")
                        off = col_off[(c, t)]
                        for s0 in range(0, Sn + 1, GCHUNK):
                            s1 = min(s0 + GCHUNK, Sn + 1)
                            nc.gpsimd.dma_gather(
                                out_ap=gt[:, s0 * RB1 : s1 * RB1].rearrange(
                                    "p (s r) -> p s r", r=RB1
                                ),
                                in_ap=hext[t * HALF : (t + 1) * HALF, :],
                                idxs_ap=sb_idx[:, off + s0 * 8 : off + s1 * 8],
                                num_idxs=(s1 - s0) * P,
                                num_idxs_reg=(s1 - s0) * P,
                                elem_size=RB1,
                            )
                        g.append(gt)
                    gA = g[0][:].bitcast(f32).rearrange("p (s r) -> p s r", r=RB1 // 2)
                    gB = g[1][:].bitcast(f32).rearrange("p (s r) -> p s r", r=RB1 // 2)

                    adst = e1w.tile([P, H], f32)
                    nc.vector.tensor_tensor(
                        out=adst[:], in0=gA[:, 0, 68:72], in1=gB[:, 0, 68:72],
                        op=mybir.AluOpType.add,
                    )
                    ST = SA + SB
                    t_all = e1w.tile([P, ST * H], f32)
                    nc.vector.tensor_tensor(
                        out=t_all[:, : SA * H].rearrange("p (s h) -> p s h", h=H),
                        in0=gA[:, 1:, 64:68],
                        in1=adst[:].unsqueeze(1).to_broadcast((P, SA, H)),
                        op=mybir.AluOpType.add,
                    )
                    nc.vector.tensor_tensor(
                        out=t_all[:, SA * H :].rearrange("p (s h) -> p s h", h=H),
                        in0=gB[:, 1:, 64:68],
                        in1=adst[:].unsqueeze(1).to_broadcast((P, SB, H)),
                        op=mybir.AluOpType.add,
                    )
                    # w = exp(leaky_relu(t, 0.2)) via fused Lrelu then Exp
                    lr_t = e1w.tile([P, ST * H], f32)
                    nc.scalar.activation(
                        lr_t[:], t_all[:], mybir.ActivationFunctionType.Lrelu, alpha=0.2
                    )
                    w_all = e1w.tile([P, ST * H], f32)
                    nc.scalar.activation(w_all[:], lr_t[:], mybir.ActivationFunctionType.Exp)
                    den = e1w.tile([P, H], f32)
                    nc.vector.tensor_reduce(
                        out=den[:],
                        in_=w_all[:].rearrange("p (s h) -> p h s", h=H),
                        axis=mybir.AxisListType.X,
                        op=mybir.AluOpType.add,
                    )
                    wb = e1w.tile([P, ST * H], bf16)
                    nc.vector.tensor_copy(wb[:], w_all[:])

                    tmp = e1t.tile([P, ST * HC], bf16)
                    nc.vector.tensor_tensor(
                        out=tmp[:, : SA * HC].rearrange("p (s h c) -> p s h c", h=H, c=C),
                        in0=g[0][:].rearrange("p (s r) -> p s r", r=RB1)[:, 1:, 0:HC]
                        .rearrange("p s (h c) -> p s h c", h=H),
                        in1=wb[:, : SA * H].rearrange("p (s h) -> p s h", h=H)
                        .unsqueeze(3).to_broadcast((P, SA, H, C)),
                        op=mybir.AluOpType.mult,
                    )
                    nc.vector.tensor_tensor(
                        out=tmp[:, SA * HC :].rearrange("p (s h c) -> p s h c", h=H, c=C),
                        in0=g[1][:].rearrange("p (s r) -> p s r", r=RB1)[:, 1:, 0:HC]
                        .rearrange("p s (h c) -> p s h c", h=H),
                        in1=wb[:, SA * H :].rearrange("p (s h) -> p s h", h=H)
                        .unsqueeze(3).to_broadcast((P, SB, H, C)),
                        op=mybir.AluOpType.mult,
                    )
                    acc = e1t.tile([P, HC], f32)
                    nc.vector.tensor_reduce(
                        out=acc[:],
                        in_=tmp[:].rearrange("p (s hc) -> p hc s", hc=HC),
                        axis=mybir.AxisListType.X,
                        op=mybir.AluOpType.add,
                    )
                    den_e = e1w.tile([P, H], f32)
                    nc.vector.tensor_scalar(
                        out=den_e[:], in0=den[:], scalar1=EPS, scalar2=None,
                        op0=mybir.AluOpType.add,
                    )
                    den_r = e1w.tile([P, H], f32)
                    nc.vector.reciprocal(den_r[:], den_e[:])
                    x2 = e1o.tile([P, HC], f32)
                    nc.vector.tensor_tensor(
                        out=x2[:].rearrange("p (h c) -> p h c", h=H),
                        in0=acc[:].rearrange("p (h c) -> p h c", h=H),
                        in1=den_r[:].unsqueeze(2).to_broadcast((P, H, C)),
                        op=mybir.AluOpType.mult,
                    )
                    nc.vector.tensor_tensor(
                        out=x2[:], in0=x2[:], in1=sb_B1[:], op=mybir.AluOpType.add
                    )
                    x2r = e1o.tile([P, HC], f32)
                    nc.scalar.activation(x2r[:], x2[:], mybir.ActivationFunctionType.Relu)

                    # layer-2 projection for this chunk
                    xt2 = e1ps2.tile([P, P], f32)
                    nc.tensor.transpose(out=xt2[:], in_=x2r[:], identity=ident_f[:])
                    nc.vector.tensor_copy(x2T_all[:, c * P : (c + 1) * P], xt2[:])
                    h2p = e1ps2.tile([P, CLASSES + 2], f32)
                    nc.tensor.matmul(
                        out=h2p[:], lhsT=x2T_all[:, c * P : (c + 1) * P], rhs=w2e[:],
                        start=True, stop=True,
                    )
                    hx2 = e1o.tile([P, RB2], f32, tag="hx2")
                    if c < 2:  # pool rotates 2 bufs; zero the tail once per buf
                        nc.gpsimd.memset(hx2[:, CLASSES + 2 : RB2], 0.0)
                    nc.vector.tensor_copy(hx2[:, 0 : CLASSES + 2], h2p[:])
                    nc.sync.dma_start(h2sh[c * P : (c + 1) * P, :], hx2[:])

                # patch local pad row asrc2 = -1e4 (every core patches its own)
                nc.sync.dma_start(
                    h2sh[PAD_LOCAL : PAD_LOCAL + 1, CLASSES : CLASSES + 1],
                    neg_const[:1, :1],
                )

            # ---------------- AllGather h2full ----------------------------
            nc.gpsimd.collective_compute(
                "AllGather",
                mybir.AluOpType.bypass,
                replica_groups=[list(range(NCORES))],
                ins=[h2sh.opt()],
                outs=[h2full.opt()],
            )

            # ---------------- layer-2 edge phase + log_softmax ------------
            with (
                tc.tile_pool(name="e2g", bufs=2) as e2g,
                tc.tile_pool(name="e2w", bufs=2) as e2w,
                tc.tile_pool(name="e2t", bufs=2) as e2t,
                tc.tile_pool(name="e2o", bufs=2) as e2o,
            ):
                for c in range(CHUNKS):
                    SA, SB = int(S[c, 0]), int(S[c, 1])
                    g = []
                    for t, Sn in ((0, SA), (1, SB)):
                        gt = e2g.tile([P, (Sn + 1) * RB2], f32, tag=f"g2{t}")
                        off = col_off[(c, t)]
                        for s0 in range(0, Sn + 1, GCHUNK):
                            s1 = min(s0 + GCHUNK, Sn + 1)
                            nc.gpsimd.dma_gather(
                                out_ap=gt[:, s0 * RB2 : s1 * RB2].rearrange(
                                    "p (s r) -> p s r", r=RB2
                                ),
                                in_ap=h2full[t * HALF : (t + 1) * HALF, :],
                                idxs_ap=sb_idx[:, off + s0 * 8 : off + s1 * 8],
                                num_idxs=(s1 - s0) * P,
                                num_idxs_reg=(s1 - s0) * P,
                                elem_size=RB2,
                            )
                        g.append(gt)
                    gA = g[0][:].rearrange("p (s r) -> p s r", r=RB2)
                    gB = g[1][:].rearrange("p (s r) -> p s r", r=RB2)

                    adst2 = e2w.tile([P, 1], f32)
                    nc.vector.tensor_tensor(
                        out=adst2[:], in0=gA[:, 0, 41:42], in1=gB[:, 0, 41:42],
                        op=mybir.AluOpType.add,
                    )
                    ST = SA + SB
                    # leaky_relu(asrc + adst2) with adst2 folded in as bias
                    lr2 = e2w.tile([P, ST], f32)
                    nc.scalar.activation(
                        lr2[:, :SA], gA[:, 1:, 40],
                        mybir.ActivationFunctionType.Lrelu,
                        bias=adst2[:, 0:1], alpha=0.2,
                    )
                    nc.scalar.activation(
                        lr2[:, SA:], gB[:, 1:, 40],
                        mybir.ActivationFunctionType.Lrelu,
                        bias=adst2[:, 0:1], alpha=0.2,
                    )
                    w2_all = e2w.tile([P, ST], f32)
                    den2 = e2w.tile([P, 1], f32)
                    nc.scalar.activation(
                        w2_all[:], lr2[:], mybir.ActivationFunctionType.Exp,
                        accum_out=den2[:, 0:1],
                    )
                    tmp2 = e2t.tile([P, ST * CLASSES], f32)
                    nc.vector.tensor_tensor(
                        out=tmp2[:, : SA * CLASSES].rearrange("p (s f) -> p s f", f=CLASSES),
                        in0=gA[:, 1:, 0:CLASSES],
                        in1=w2_all[:, :SA].unsqueeze(2).to_broadcast((P, SA, CLASSES)),
                        op=mybir.AluOpType.mult,
                    )
                    nc.vector.tensor_tensor(
                        out=tmp2[:, SA * CLASSES :].rearrange("p (s f) -> p s f", f=CLASSES),
                        in0=gB[:, 1:, 0:CLASSES],
                        in1=w2_all[:, SA:].unsqueeze(2).to_broadcast((P, SB, CLASSES)),
                        op=mybir.AluOpType.mult,
                    )
                    acc2 = e2t.tile([P, CLASSES], f32)
                    nc.vector.tensor_reduce(
                        out=acc2[:],
                        in_=tmp2[:].rearrange("p (s f) -> p f s", f=CLASSES),
                        axis=mybir.AxisListType.X,
                        op=mybir.AluOpType.add,
                    )
                    den2e = e2w.tile([P, 1], f32)
                    nc.vector.tensor_scalar(
                        out=den2e[:], in0=den2[:], scalar1=EPS, scalar2=None,
                        op0=mybir.AluOpType.add,
                    )
                    den2r = e2w.tile([P, 1], f32)
                    nc.vector.reciprocal(den2r[:], den2e[:])
                    o_pre = e2o.tile([P, CLASSES], f32)
                    nc.vector.tensor_tensor(
                        out=o_pre[:], in0=acc2[:],
                        in1=den2r[:].to_broadcast((P, CLASSES)),
                        op=mybir.AluOpType.mult,
                    )
                    nc.vector.tensor_tensor(
                        out=o_pre[:], in0=o_pre[:], in1=sb_B2[:], op=mybir.AluOpType.add
                    )
                    # log_softmax
                    nmax = e2w.tile([P, 1], f32)
                    nc.vector.tensor_reduce(
                        out=nmax[:], in_=o_pre[:], axis=mybir.AxisListType.X,
                        op=mybir.AluOpType.max, negate=True,
                    )
                    expt = e2w.tile([P, CLASSES], f32)
                    sumexp = e2w.tile([P, 1], f32)
                    nc.scalar.activation(
                        expt[:], o_pre[:], mybir.ActivationFunctionType.Exp,
                        bias=nmax[:, 0:1], accum_out=sumexp[:, 0:1],
                    )
                    lse = e2w.tile([P, 1], f32)
                    nc.scalar.activation(lse[:], sumexp[:], mybir.ActivationFunctionType.Ln)
                    sh = e2w.tile([P, 1], f32)
                    nc.vector.tensor_tensor(
                        out=sh[:], in0=nmax[:], in1=lse[:], op=mybir.AluOpType.subtract
                    )
                    o_f = e2o.tile([P, CLASSES], bf16)
                    nc.scalar.activation(
                        o_f[:], o_pre[:], mybir.ActivationFunctionType.Identity,
                        bias=sh[:, 0:1],
                    )
                    nc.sync.dma_start(o_out[c * P : (c + 1) * P, :], o_f[:])
    nc.finalize()
    return nc


def _kernel_impl(x, W1, a_src1, a_dst1, b1, W2, a_src2, a_dst2, b2, edge_index, cfg):
    import time as _time

    _tm = os.environ.get("K_TIMING", "0") == "1"
    _t0 = _time.time()

    def _lap(tag):
        nonlocal _t0
        if _tm:
            t = _time.time()
            print(f"[k] {tag}: {t - _t0:.2f}s", flush=True)
            _t0 = t

    N, F, H, C, CLASSES, NCORES = (
        cfg["N"], cfg["F"], cfg["H"], cfg["C"], cfg["CLASSES"], cfg["NCORES"]
    )
    HC = H * C
    x = np.asarray(x, dtype=np.float32)
    band, meta = _host_tables(np.asarray(edge_index), cfg)
    NPC, NTOT = meta["NPC"], meta["NTOT"]
    row_of = meta["row_of"]
    _lap("host tables")

    # numpy input prep runs in a worker thread, overlapped with build_program
    prep = {}

    def _prep_inputs():
        xTb = np.zeros((F, NTOT), dtype=ml_dtypes.bfloat16)
        xTb[:, row_of] = np.asarray(x, np.float32).T.astype(ml_dtypes.bfloat16)
        a1s = np.asarray(a_src1, np.float32)
        a1d = np.asarray(a_dst1, np.float32)
        A1 = np.zeros((HC, 2 * H), dtype=np.float32)
        for h in range(H):
            A1[h * C : (h + 1) * C, h] = a1s[h]
            A1[h * C : (h + 1) * C, H + h] = a1d[h]
        A2 = np.stack(
            [np.asarray(a_src2, np.float32)[0], np.asarray(a_dst2, np.float32)[0]],
            axis=1,
        )
        W1f = np.asarray(W1, np.float32)
        W2f = np.asarray(W2, np.float32)
        common = dict(
            W1e=np.concatenate([W1f, W1f @ A1], axis=1).astype(ml_dtypes.bfloat16),
            W2e=np.concatenate([W2f, W2f @ A2], axis=1),
            B1=np.ascontiguousarray(np.broadcast_to(np.asarray(b1, np.float32), (128, HC))),
            B2=np.ascontiguousarray(
                np.broadcast_to(np.asarray(b2, np.float32), (128, CLASSES))
            ),
        )
        prep["in_maps"] = [
            dict(
                common,
                idx=band[k],
                xT=np.ascontiguousarray(xTb[:, k * NPC : (k + 1) * NPC]),
            )
            for k in range(NCORES)
        ]

    prep_t = threading.Thread(target=_prep_inputs, daemon=True)
    prep_t.start()

    for _t in _warm_threads:
        _t.join()
    _lap("warm join")
    from concourse.bass_utils import run_bass_kernel_spmd

    nc = _build_program(cfg, meta)
    _lap("build_program")
    prep_t.join()
    in_maps = prep["in_maps"]
    _lap("in_maps join")
    res = run_bass_kernel_spmd(nc, in_maps, list(range(NCORES)))
    _lap("run")
    outs = np.concatenate(
        [np.asarray(res.results[k]["out"]) for k in range(NCORES)], axis=0
    ).astype(np.float32)
    _lap("gather out")
    return np.ascontiguousarray(outs[row_of])


def kernel(x, W1, a_src1, a_dst1, b1, W2, a_src2, a_dst2, b2, edge_index):
    return _kernel_impl(
        x, W1, a_src1, a_dst1, b1, W2, a_src2, a_dst2, b2, edge_index, _default_cfg()
    )
